# revision 19
# baseline (speedup 1.0000x reference)
"""Trainium2 Bass kernel for a BERT-style feed-forward expert with LoRA in/out
adapters and a final no-affine layernorm.

Reference computation (per token t, D=1024, H=4096, R=8):
    lora_up  = (x @ A_up.T) @ B_up.T * s
    hidden   = gelu(x @ Wi.T + bi) + lora_up
    out      = layernorm(base + (hidden @ A_down.T) @ B_down.T * s)

Structural facts exploited:
  * `hidden` is consumed ONLY through the rank-8 down projection, so the
    lora_up contribution collapses algebraically:
        r2 = A_down @ hidden.T
           = A_down @ gelu(z).T + (s * A_down @ B_up @ A_up) @ x.T
    The 8x1024 matrix A_fold = s * A_down @ B_up @ A_up is precomputed on the
    host; B_up never reaches the device.
  * The final output is LN(base + r2.T @ (s*B_down.T)) with the rank-8 term a
    small perturbation of base, so bf16 matmuls are far more than accurate
    enough; the base-add and layernorm run in fp32.

Device dataflow per core (2048 tokens, data-parallel, no collectives):
  * x^T and Wi^T arrive host-pre-transposed so the D contraction sits on SBUF
    partitions; hidden is produced as [H, tok] tiles and consumed immediately
    by the r2 accumulation - never materialized.
  * All rank-8 matmuls (M=8) are column-packed 4-wide into PE column strips
    via tile_position, so 4 of them cost ~1 matmul of time.
  * The PE instruction stream is kept stall-free by emission-time software
    pipelining: each r2 strip group is emitted two H-blocks after the gelus
    it consumes, and stage C (down-proj + base-add + layernorm) of tile t is
    spread one subtile per H-block across tile t+1.
  * rsqrt for layernorm is a Quake-seeded Newton iteration on the DVE -
    keeping Sqrt off the Scalar engine avoids Gelu<->Sqrt ACT-table thrash.
"""

from contextlib import ExitStack

import numpy as np
import ml_dtypes

import concourse.mybir as mybir
import concourse.tile as tile
from concourse import bacc
from concourse.bass_utils import run_bass_kernel_spmd

# Problem shape (hardcoded per contract; kernel.py must be self-contained).
B, S, D, H, R = 4, 4096, 1024, 4096, 8
NCORES = 8
TOK = (B * S) // NCORES      # tokens per core = 2048
TT = 512                     # token tile (matmul moving free dim)
NTT = TOK // TT              # 4 token tiles
P = 128
KO = D // P                  # 8  k-subtiles of the D contraction
HO = H // P                  # 32 hidden chunks
HB = HO // 4                 # 8  blocks of 4 hidden chunks
NSB = TT // P                # 4  token subtiles per tile
SCALING = 16.0 / 8.0         # lora_alpha / lora_r
LN_EPS = 1e-5

BF16 = mybir.dt.bfloat16
F32 = mybir.dt.float32
I32 = mybir.dt.int32

RSQRT_MAGIC = 0x5F3759DF

_NC_CACHE = {}
last_results = None          # test harness reads exec_time_ns from here

# CoreSim does not implement Gelu; sim_check.py overrides this with Tanh to
# validate scheduling/layout. Hardware always runs the real (exact) Gelu.
ACT_FUNC = mybir.ActivationFunctionType.Gelu


def _build_nc():
    nc = bacc.Bacc("TRN2", target_bir_lowering=False, debug=False,
                   num_devices=NCORES)

    xT = nc.dram_tensor("xT", [D, TOK], BF16, kind="ExternalInput").ap()
    base = nc.dram_tensor("base", [TOK, D], F32, kind="ExternalInput").ap()
    WiT = nc.dram_tensor("WiT", [D, H], BF16, kind="ExternalInput").ap()
    bi = nc.dram_tensor("bi", [H], F32, kind="ExternalInput").ap()
    AfT = nc.dram_tensor("AfT", [D, R], BF16, kind="ExternalInput").ap()
    AdT = nc.dram_tensor("AdT", [H, R], BF16, kind="ExternalInput").ap()
    BdT = nc.dram_tensor("BdT", [R, D], BF16, kind="ExternalInput").ap()
    out = nc.dram_tensor("out", [TOK, D], F32, kind="ExternalOutput").ap()

    with tile.TileContext(nc) as tc, ExitStack() as ctx:
        _body(tc, ctx, xT, base, WiT, bi, AfT, AdT, BdT, out)
    nc.compile()
    return nc


def _body(tc, ctx, xT, base, WiT, bi, AfT, AdT, BdT, out):
    nc = tc.nc
    assert nc.vector.BN_STATS_FMAX >= 512

    singles = ctx.enter_context(tc.tile_pool(name="singles", bufs=1))
    gpool = ctx.enter_context(tc.tile_pool(name="gpool", bufs=20))
    rpool = ctx.enter_context(tc.tile_pool(name="rpool", bufs=2))
    bpool = ctx.enter_context(tc.tile_pool(name="bpool", bufs=3))
    ypool = ctx.enter_context(tc.tile_pool(name="ypool", bufs=6))
    opool = ctx.enter_context(tc.tile_pool(name="opool", bufs=3))
    spool = ctx.enter_context(tc.tile_pool(name="spool", bufs=4))
    # PSUM banks: 3 (hidden chunks) + 2 (r2 accumulators) + 2 + 1 (down-proj
    # halves) = 8
    psum_h = ctx.enter_context(tc.tile_pool(name="psum_h", bufs=3, space="PSUM"))
    psum_r2 = ctx.enter_context(tc.tile_pool(name="psum_r2", bufs=2, space="PSUM"))
    psum_d0 = ctx.enter_context(tc.tile_pool(name="psum_d0", bufs=2, space="PSUM"))
    psum_d1 = ctx.enter_context(tc.tile_pool(name="psum_d1", bufs=1, space="PSUM"))

    gelu = ACT_FUNC
    sub = mybir.AluOpType.subtract
    mult = mybir.AluOpType.mult
    add = mybir.AluOpType.add
    shr = mybir.AluOpType.arith_shift_right

    # ---- resident tensors; emission order = DMA priority. The very first
    # compute is main block b0 (ho 0..3): its first weights chunk goes first,
    # then x tile 0 (split across queues), then everything else.
    wiT_sb = singles.tile([P, KO, H], BF16)
    wiT_dram = WiT.rearrange("(ko p) h -> p ko h", p=P)
    xT_sb = singles.tile([P, KO, TOK], BF16)
    xT_dram = xT.rearrange("(ko p) t -> p ko t", p=P)

    nc.sync.dma_start(wiT_sb[:, :, 0:P], wiT_dram[:, :, 0:P])
    nc.sync.dma_start(xT_sb[:, :, 0:TT], xT_dram[:, :, 0:TT])
    nc.sync.dma_start(wiT_sb[:, :, P:512], wiT_dram[:, :, P:512])
    nc.sync.dma_start(wiT_sb[:, :, 512:1024], wiT_dram[:, :, 512:1024])

    bi_sb = singles.tile([P, HO], F32)
    nc.sync.dma_start(bi_sb, bi.rearrange("(ho p) -> p ho", p=P))
    afT_sb = singles.tile([P, KO, R], BF16)
    nc.sync.dma_start(afT_sb, AfT.rearrange("(ko p) r -> p ko r", p=P))
    adT_sb = singles.tile([P, HO, R], BF16)
    nc.sync.dma_start(adT_sb, AdT.rearrange("(ho p) r -> p ho r", p=P))
    bdT_sb = singles.tile([R, D], BF16)
    nc.sync.dma_start(bdT_sb, BdT)

    HC = 512  # H-chunks; chunk hc holds the weights for hidden block hb=hc
    for hc in range(2, H // HC):
        nc.sync.dma_start(wiT_sb[:, :, hc * HC:(hc + 1) * HC],
                          wiT_dram[:, :, hc * HC:(hc + 1) * HC])
        if hc - 1 < NTT:
            nc.sync.dma_start(xT_sb[:, :, (hc - 1) * TT:hc * TT],
                              xT_dram[:, :, (hc - 1) * TT:hc * TT])

    tile_state = {}

    def emit_r2_group(t2, hb2):
        # 4 rank-8 matmuls packed into distinct PE column strips; they run
        # concurrently in the array (separate col groups / XBUSes).
        st = tile_state[t2]
        pr2 = st["pr2"]
        last = hb2 == HB - 1
        for g, ho in st["groups"][hb2]:
            j = ho % 4
            nc.tensor.matmul(pr2[32 * j:32 * j + R, :], adT_sb[:, ho, :], g,
                             start=False, stop=last, tile_position=(0, 32 * j),
                             skip_group_check=True)
        st["groups"][hb2] = None
        if last:
            # combine the 4 column strips -> r2 [8, TT] (bf16 for down-proj),
            # one 128-token slice at a time so the first down-proj matmul can
            # start after ~1/4 of the combine work
            s0 = rpool.tile([R, TT], F32, name="s0")
            r2_sb = rpool.tile([R, TT], BF16, name="r2_sb")
            for sb in range(NSB):
                c = slice(sb * P, (sb + 1) * P)
                nc.vector.tensor_copy(s0[:, c], pr2[0:R, c])
                nc.vector.tensor_add(s0[:, c], s0[:, c], pr2[32:32 + R, c])
                nc.vector.tensor_add(s0[:, c], s0[:, c], pr2[64:64 + R, c])
                nc.vector.tensor_add(r2_sb[:, c], s0[:, c], pr2[96:96 + R, c])
            st["r2"] = r2_sb

    def stage_c_sub(st, sb):
        # down-projection + base add + layernorm stats for one 128-token
        # subtile of token tile st["t"]
        t0 = st["t"] * TT + sb * P
        r2_sb = st["r2"]
        pd0 = psum_d0.tile([P, 512], F32)
        pd1 = psum_d1.tile([P, 512], F32)
        nc.tensor.matmul(pd0, r2_sb[:, sb * P:(sb + 1) * P],
                         bdT_sb[:, 0:512], start=True, stop=True)
        nc.tensor.matmul(pd1, r2_sb[:, sb * P:(sb + 1) * P],
                         bdT_sb[:, 512:1024], start=True, stop=True)
        bt = bpool.tile([P, D], F32)
        nc.sync.dma_start(bt, base[t0:t0 + P, :])
        y = ypool.tile([P, D], F32)
        nc.vector.tensor_add(y[:, 0:512], bt[:, 0:512], pd0)
        nc.vector.tensor_add(y[:, 512:1024], bt[:, 512:1024], pd1)
        stats = spool.tile([P, 2, nc.vector.BN_STATS_DIM], F32)
        nc.vector.bn_stats(stats[:, 0, :], y[:, 0:512])
        nc.vector.bn_stats(stats[:, 1, :], y[:, 512:1024])
        nc.vector.bn_aggr(st["mvs"][:, sb, :], stats)
        st["ys"].append(y)

    def stage_c_fin(st):
        # rstd = rsqrt(var + eps) for all 4 subtiles at once, DVE-only:
        # Quake-III seed then 2 Newton iterations (error ~4e-6, far below
        # the bf16 matmul error floor).
        mvs = st["mvs"]
        v = spool.tile([P, NSB], F32)
        nc.vector.tensor_scalar_add(v, mvs[:, :, 1], LN_EPS)
        iv = spool.tile([P, NSB], I32)
        nc.vector.tensor_scalar(out=iv, in0=v.bitcast(I32), scalar1=1,
                                scalar2=None, op0=shr)
        nc.vector.tensor_scalar(out=iv, in0=iv, scalar1=-1, scalar2=RSQRT_MAGIC,
                                op0=mult, op1=add)
        r = iv.bitcast(F32)
        tmp = spool.tile([P, NSB], F32)
        for _ in range(2):
            nc.vector.tensor_mul(tmp, v, r)
            nc.vector.tensor_mul(tmp, tmp, r)
            nc.vector.tensor_scalar(out=tmp, in0=tmp, scalar1=-0.5, scalar2=1.5,
                                    op0=mult, op1=add)
            nc.vector.tensor_mul(r, r, tmp)

        for sb in range(NSB):
            t0 = st["t"] * TT + sb * P
            o = opool.tile([P, D], F32)
            # GpSimd (otherwise idle) so the normalize overlaps DVE work
            nc.gpsimd.tensor_scalar(out=o, in0=st["ys"][sb],
                                    scalar1=mvs[:, sb, 0:1],
                                    scalar2=r[:, sb:sb + 1], op0=sub, op1=mult)
            nc.sync.dma_start(out[t0:t0 + P, :], o)

    def tick(gb):
        # emission-time pipeline: r2 strip groups run 2 blocks behind their
        # gelus; stage C of tile t runs one subtile per block across tile t+1
        g2 = gb - 2
        if g2 >= 0:
            t2, hb2 = divmod(g2, HB)
            if t2 < NTT:
                emit_r2_group(t2, hb2)
        g3 = gb - (HB + 2)
        if g3 >= 0:
            t3, k = divmod(g3, HB)
            if t3 < NTT:
                if k < NSB:
                    stage_c_sub(tile_state[t3], k)
                elif k == NSB:
                    stage_c_fin(tile_state[t3])

    for t in range(NTT):
        tsl = slice(t * TT, (t + 1) * TT)
        # r2[r, tok] accumulates in one PSUM bank split into 4 column strips:
        # the folded lora-up path (8 D-chunks) starts the strips, then the
        # 32 gelu H-chunks accumulate into them.
        st = {"t": t,
              "pr2": psum_r2.tile([P, TT], F32, name="pr2"),
              "groups": [], "ys": [], "r2": None,
              "mvs": spool.tile([P, NSB, nc.vector.BN_AGGR_DIM], F32,
                                name="mvs")}
        tile_state[t] = st

        for hb in range(HB):
            group = []
            for hj in range(4):
                ho = hb * 4 + hj
                ph = psum_h.tile([P, TT], F32)
                for ko in range(KO):
                    nc.tensor.matmul(ph, wiT_sb[:, ko, ho * P:(ho + 1) * P],
                                     xT_sb[:, ko, tsl],
                                     start=(ko == 0), stop=(ko == KO - 1))
                g = gpool.tile([P, TT], BF16)
                nc.scalar.activation(g, ph, gelu, bias=bi_sb[:, ho:ho + 1],
                                     scale=1.0)
                group.append((g, ho))
            st["groups"].append(group)
            if hb == 0:
                pr2 = st["pr2"]
                for ko in range(KO):
                    j = ko % 4
                    nc.tensor.matmul(pr2[32 * j:32 * j + R, :], afT_sb[:, ko, :],
                                     xT_sb[:, ko, tsl],
                                     start=(ko < 4), stop=False,
                                     tile_position=(0, 32 * j),
                                     skip_group_check=True)
            tick(t * HB + hb)

    for gb in range(NTT * HB, NTT * HB + HB + NSB + 3):
        tick(gb)


def _get_nc():
    if "nc" not in _NC_CACHE:
        _NC_CACHE["nc"] = _build_nc()
    return _NC_CACHE["nc"]


def kernel(x, base_output, Wi, bi, A_up, B_up, A_down, B_down):
    global last_results
    bf = ml_dtypes.bfloat16

    x2 = np.asarray(x, dtype=np.float32).reshape(B * S, D)
    base2 = np.asarray(base_output, dtype=np.float32).reshape(B * S, D)

    A_up64 = np.asarray(A_up, dtype=np.float64)
    B_up64 = np.asarray(B_up, dtype=np.float64)
    A_down64 = np.asarray(A_down, dtype=np.float64)
    A_fold = (SCALING * (A_down64 @ B_up64)) @ A_up64          # [R, D]

    AfT = np.ascontiguousarray(A_fold.T).astype(bf)            # [D, R]
    AdT = np.ascontiguousarray(A_down64.T).astype(bf)          # [H, R]
    BdT = np.ascontiguousarray(
        (SCALING * np.asarray(B_down, np.float64)).T).astype(bf)  # [R, D]
    WiT_b = np.ascontiguousarray(np.asarray(Wi, np.float32).T).astype(bf)  # [D, H]
    bi32 = np.ascontiguousarray(np.asarray(bi, np.float32))

    in_maps = []
    for c in range(NCORES):
        rows = slice(c * TOK, (c + 1) * TOK)
        in_maps.append(dict(
            xT=np.ascontiguousarray(x2[rows].T).astype(bf),
            base=np.ascontiguousarray(base2[rows]),
            WiT=WiT_b, bi=bi32, AfT=AfT, AdT=AdT, BdT=BdT,
        ))

    nc = _get_nc()
    last_results = run_bass_kernel_spmd(nc, in_maps, core_ids=list(range(NCORES)))
    out = np.concatenate([r["out"] for r in last_results.results], axis=0)
    return np.ascontiguousarray(out.reshape(B, S, D).astype(np.float32))


# revision 20
# speedup vs baseline: 1.3977x; 1.3977x over previous
"""Trainium2 Bass kernel for a BERT-style feed-forward expert with LoRA in/out
adapters and a final no-affine layernorm.

Reference computation (per token t, D=1024, H=4096, R=8):
    lora_up  = (x @ A_up.T) @ B_up.T * s
    hidden   = gelu(x @ Wi.T + bi) + lora_up
    out      = layernorm(base + (hidden @ A_down.T) @ B_down.T * s)

Structural facts exploited:
  * `hidden` is consumed ONLY through the rank-8 down projection, so the
    lora_up contribution collapses algebraically:
        r2 = A_down @ hidden.T
           = A_down @ gelu(z).T + (s * A_down @ B_up @ A_up) @ x.T
    The 8x1024 matrix A_fold = s * A_down @ B_up @ A_up is precomputed on the
    host; B_up never reaches the device.
  * The final output is LN(base + r2.T @ (s*B_down.T)) with the rank-8 term a
    small perturbation of base, so bf16 matmuls are far more than accurate
    enough; the base-add and layernorm run in fp32.

Device dataflow per core (2048 tokens, data-parallel, no collectives):
  * x^T and Wi^T arrive host-pre-transposed so the D contraction sits on SBUF
    partitions; hidden is produced as [H, tok] tiles and consumed immediately
    by the r2 accumulation - never materialized.
  * All rank-8 matmuls (M=8) are column-packed 4-wide into PE column strips
    via tile_position, so 4 of them cost ~1 matmul of time.
  * The PE instruction stream is kept stall-free by emission-time software
    pipelining: each r2 strip group is emitted two H-blocks after the gelus
    it consumes, and stage C (down-proj + base-add + layernorm) of tile t is
    spread one subtile per H-block across tile t+1.
  * rsqrt for layernorm is a Quake-seeded Newton iteration on the DVE -
    keeping Sqrt off the Scalar engine avoids Gelu<->Sqrt ACT-table thrash.
"""

from contextlib import ExitStack

import numpy as np
import ml_dtypes

import concourse.mybir as mybir
import concourse.tile as tile
from concourse import bacc
from concourse.bass_utils import run_bass_kernel_spmd

# Problem shape (hardcoded per contract; kernel.py must be self-contained).
B, S, D, H, R = 4, 4096, 1024, 4096, 8
NCORES = 8
TOK = (B * S) // NCORES      # tokens per core = 2048
TT = 512                     # token tile (matmul moving free dim)
NTT = TOK // TT              # 4 token tiles
P = 128
KO = D // P                  # 8  k-subtiles of the D contraction
HO = H // P                  # 32 hidden chunks
HB = HO // 4                 # 8  blocks of 4 hidden chunks
NSB = TT // P                # 4  token subtiles per tile
SCALING = 16.0 / 8.0         # lora_alpha / lora_r
LN_EPS = 1e-5

BF16 = mybir.dt.bfloat16
F32 = mybir.dt.float32
I32 = mybir.dt.int32

RSQRT_MAGIC = 0x5F3759DF

_NC_CACHE = {}
last_results = None          # test harness reads exec_time_ns from here

# CoreSim does not implement Gelu; sim_check.py overrides this with Tanh to
# validate scheduling/layout. Hardware always runs the real (exact) Gelu.
ACT_FUNC = mybir.ActivationFunctionType.Gelu


def _build_nc():
    nc = bacc.Bacc("TRN2", target_bir_lowering=False, debug=False,
                   num_devices=NCORES)

    xT = nc.dram_tensor("xT", [D, TOK], BF16, kind="ExternalInput").ap()
    base = nc.dram_tensor("base", [TOK, D], F32, kind="ExternalInput").ap()
    WiT = nc.dram_tensor("WiT", [D, H], BF16, kind="ExternalInput").ap()
    bi = nc.dram_tensor("bi", [H], F32, kind="ExternalInput").ap()
    AfT = nc.dram_tensor("AfT", [D, R], BF16, kind="ExternalInput").ap()
    AdT = nc.dram_tensor("AdT", [H, R], BF16, kind="ExternalInput").ap()
    BdT = nc.dram_tensor("BdT", [R, D], BF16, kind="ExternalInput").ap()
    out = nc.dram_tensor("out", [TOK, D], F32, kind="ExternalOutput").ap()

    with tile.TileContext(nc) as tc, ExitStack() as ctx:
        _body(tc, ctx, xT, base, WiT, bi, AfT, AdT, BdT, out)
    nc.compile()
    return nc


def _body(tc, ctx, xT, base, WiT, bi, AfT, AdT, BdT, out):
    nc = tc.nc
    assert nc.vector.BN_STATS_FMAX >= 512

    singles = ctx.enter_context(tc.tile_pool(name="singles", bufs=1))
    gpool = ctx.enter_context(tc.tile_pool(name="gpool", bufs=20))
    rpool = ctx.enter_context(tc.tile_pool(name="rpool", bufs=2))
    bpool = ctx.enter_context(tc.tile_pool(name="bpool", bufs=3))
    ypool = ctx.enter_context(tc.tile_pool(name="ypool", bufs=6))
    opool = ctx.enter_context(tc.tile_pool(name="opool", bufs=3))
    spool = ctx.enter_context(tc.tile_pool(name="spool", bufs=4))
    # PSUM banks: 3 (hidden chunks) + 2 (r2 accumulators) + 2 + 1 (down-proj
    # halves) = 8
    psum_h = ctx.enter_context(tc.tile_pool(name="psum_h", bufs=3, space="PSUM"))
    psum_r2 = ctx.enter_context(tc.tile_pool(name="psum_r2", bufs=2, space="PSUM"))
    psum_d0 = ctx.enter_context(tc.tile_pool(name="psum_d0", bufs=2, space="PSUM"))
    psum_d1 = ctx.enter_context(tc.tile_pool(name="psum_d1", bufs=1, space="PSUM"))

    gelu = ACT_FUNC
    sub = mybir.AluOpType.subtract
    mult = mybir.AluOpType.mult
    add = mybir.AluOpType.add
    shr = mybir.AluOpType.arith_shift_right

    # ---- resident tensors; emission order = DMA priority. The very first
    # compute is main block b0 (ho 0..3): its first weights chunk goes first,
    # then x tile 0 (split across queues), then everything else.
    wiT_sb = singles.tile([P, KO, H], BF16)
    wiT_dram = WiT.rearrange("(ko p) h -> p ko h", p=P)
    xT_sb = singles.tile([P, KO, TOK], BF16)
    xT_dram = xT.rearrange("(ko p) t -> p ko t", p=P)

    nc.sync.dma_start(wiT_sb[:, :, 0:P], wiT_dram[:, :, 0:P])
    nc.sync.dma_start(xT_sb[:, :, 0:TT], xT_dram[:, :, 0:TT])
    nc.sync.dma_start(wiT_sb[:, :, P:512], wiT_dram[:, :, P:512])
    nc.sync.dma_start(wiT_sb[:, :, 512:1024], wiT_dram[:, :, 512:1024])

    bi_sb = singles.tile([P, HO], F32)
    nc.sync.dma_start(bi_sb, bi.rearrange("(ho p) -> p ho", p=P))
    afT_sb = singles.tile([P, KO, R], BF16)
    nc.sync.dma_start(afT_sb, AfT.rearrange("(ko p) r -> p ko r", p=P))
    adT_sb = singles.tile([P, HO, R], BF16)
    nc.sync.dma_start(adT_sb, AdT.rearrange("(ho p) r -> p ho r", p=P))
    bdT_sb = singles.tile([R, D], BF16)
    nc.sync.dma_start(bdT_sb, BdT)

    HC = 512  # H-chunks; chunk hc holds the weights for hidden block hb=hc
    for hc in range(2, H // HC):
        nc.sync.dma_start(wiT_sb[:, :, hc * HC:(hc + 1) * HC],
                          wiT_dram[:, :, hc * HC:(hc + 1) * HC])
        if hc - 1 < NTT:
            nc.sync.dma_start(xT_sb[:, :, (hc - 1) * TT:hc * TT],
                              xT_dram[:, :, (hc - 1) * TT:hc * TT])

    tile_state = {}

    def emit_r2_group(t2, hb2):
        # 4 rank-8 matmuls packed into distinct PE column strips; they run
        # concurrently in the array (separate col groups / XBUSes).
        st = tile_state[t2]
        pr2 = st["pr2"]
        last = hb2 == HB - 1
        for g, ho in st["groups"][hb2]:
            j = ho % 4
            nc.tensor.matmul(pr2[32 * j:32 * j + R, :], adT_sb[:, ho, :], g,
                             start=False, stop=last, tile_position=(0, 32 * j),
                             skip_group_check=True)
        st["groups"][hb2] = None
        if last:
            # combine the 4 column strips -> r2 [8, TT] (bf16 for down-proj),
            # one 128-token slice at a time so the first down-proj matmul can
            # start after ~1/4 of the combine work
            s0 = rpool.tile([R, TT], F32, name="s0")
            r2_sb = rpool.tile([R, TT], BF16, name="r2_sb")
            for sb in range(NSB):
                c = slice(sb * P, (sb + 1) * P)
                nc.vector.tensor_copy(s0[:, c], pr2[0:R, c])
                nc.vector.tensor_add(s0[:, c], s0[:, c], pr2[32:32 + R, c])
                nc.vector.tensor_add(s0[:, c], s0[:, c], pr2[64:64 + R, c])
                nc.vector.tensor_add(r2_sb[:, c], s0[:, c], pr2[96:96 + R, c])
            st["r2"] = r2_sb

    def stage_c_sub(st, sb):
        # down-projection + base add + layernorm stats for one 128-token
        # subtile of token tile st["t"]
        t0 = st["t"] * TT + sb * P
        r2_sb = st["r2"]
        pd0 = psum_d0.tile([P, 512], F32)
        pd1 = psum_d1.tile([P, 512], F32)
        nc.tensor.matmul(pd0, r2_sb[:, sb * P:(sb + 1) * P],
                         bdT_sb[:, 0:512], start=True, stop=True)
        nc.tensor.matmul(pd1, r2_sb[:, sb * P:(sb + 1) * P],
                         bdT_sb[:, 512:1024], start=True, stop=True)
        bt = bpool.tile([P, D], F32)
        nc.sync.dma_start(bt, base[t0:t0 + P, :])
        y = ypool.tile([P, D], F32)
        nc.vector.tensor_add(y[:, 0:512], bt[:, 0:512], pd0)
        nc.vector.tensor_add(y[:, 512:1024], bt[:, 512:1024], pd1)
        stats = spool.tile([P, 2, nc.vector.BN_STATS_DIM], F32)
        nc.vector.bn_stats(stats[:, 0, :], y[:, 0:512])
        nc.vector.bn_stats(stats[:, 1, :], y[:, 512:1024])
        nc.vector.bn_aggr(st["mvs"][:, sb, :], stats)
        st["ys"].append(y)

    def stage_c_fin(st):
        # rstd = rsqrt(var + eps) for all 4 subtiles at once, DVE-only:
        # Quake-III seed then 2 Newton iterations (error ~4e-6, far below
        # the bf16 matmul error floor).
        mvs = st["mvs"]
        v = spool.tile([P, NSB], F32)
        nc.vector.tensor_scalar_add(v, mvs[:, :, 1], LN_EPS)
        iv = spool.tile([P, NSB], I32)
        nc.vector.tensor_scalar(out=iv, in0=v.bitcast(I32), scalar1=1,
                                scalar2=None, op0=shr)
        nc.vector.tensor_scalar(out=iv, in0=iv, scalar1=-1, scalar2=RSQRT_MAGIC,
                                op0=mult, op1=add)
        r = iv.bitcast(F32)
        tmp = spool.tile([P, NSB], F32)
        for _ in range(2):
            nc.vector.tensor_mul(tmp, v, r)
            nc.vector.tensor_mul(tmp, tmp, r)
            nc.vector.tensor_scalar(out=tmp, in0=tmp, scalar1=-0.5, scalar2=1.5,
                                    op0=mult, op1=add)
            nc.vector.tensor_mul(r, r, tmp)

        for sb in range(NSB):
            t0 = st["t"] * TT + sb * P
            o = opool.tile([P, D], F32)
            nc.vector.tensor_scalar(out=o, in0=st["ys"][sb],
                                    scalar1=mvs[:, sb, 0:1],
                                    scalar2=r[:, sb:sb + 1], op0=sub, op1=mult)
            nc.sync.dma_start(out[t0:t0 + P, :], o)

    def tick(gb):
        # emission-time pipeline: r2 strip groups run 2 blocks behind their
        # gelus; stage C of tile t runs one subtile per block across tile t+1
        g2 = gb - 2
        if g2 >= 0:
            t2, hb2 = divmod(g2, HB)
            if t2 < NTT:
                emit_r2_group(t2, hb2)
        g3 = gb - (HB + 2)
        if g3 >= 0:
            t3, k = divmod(g3, HB)
            if t3 < NTT:
                if k < NSB:
                    stage_c_sub(tile_state[t3], k)
                elif k == NSB:
                    stage_c_fin(tile_state[t3])

    for t in range(NTT):
        tsl = slice(t * TT, (t + 1) * TT)
        # r2[r, tok] accumulates in one PSUM bank split into 4 column strips:
        # the folded lora-up path (8 D-chunks) starts the strips, then the
        # 32 gelu H-chunks accumulate into them.
        st = {"t": t,
              "pr2": psum_r2.tile([P, TT], F32, name="pr2"),
              "groups": [], "ys": [], "r2": None,
              "mvs": spool.tile([P, NSB, nc.vector.BN_AGGR_DIM], F32,
                                name="mvs")}
        tile_state[t] = st

        for hb in range(HB):
            group = []
            for hj in range(4):
                ho = hb * 4 + hj
                ph = psum_h.tile([P, TT], F32)
                for ko in range(KO):
                    nc.tensor.matmul(ph, wiT_sb[:, ko, ho * P:(ho + 1) * P],
                                     xT_sb[:, ko, tsl],
                                     start=(ko == 0), stop=(ko == KO - 1))
                g = gpool.tile([P, TT], BF16)
                nc.scalar.activation(g, ph, gelu, bias=bi_sb[:, ho:ho + 1],
                                     scale=1.0)
                group.append((g, ho))
            st["groups"].append(group)
            if hb == 0:
                pr2 = st["pr2"]
                for ko in range(KO):
                    j = ko % 4
                    nc.tensor.matmul(pr2[32 * j:32 * j + R, :], afT_sb[:, ko, :],
                                     xT_sb[:, ko, tsl],
                                     start=(ko < 4), stop=False,
                                     tile_position=(0, 32 * j),
                                     skip_group_check=True)
            tick(t * HB + hb)

    for gb in range(NTT * HB, NTT * HB + HB + NSB + 3):
        tick(gb)


def _get_nc():
    if "nc" not in _NC_CACHE:
        _NC_CACHE["nc"] = _build_nc()
    return _NC_CACHE["nc"]


def kernel(x, base_output, Wi, bi, A_up, B_up, A_down, B_down):
    global last_results
    bf = ml_dtypes.bfloat16

    x2 = np.asarray(x, dtype=np.float32).reshape(B * S, D)
    base2 = np.asarray(base_output, dtype=np.float32).reshape(B * S, D)

    A_up64 = np.asarray(A_up, dtype=np.float64)
    B_up64 = np.asarray(B_up, dtype=np.float64)
    A_down64 = np.asarray(A_down, dtype=np.float64)
    A_fold = (SCALING * (A_down64 @ B_up64)) @ A_up64          # [R, D]

    AfT = np.ascontiguousarray(A_fold.T).astype(bf)            # [D, R]
    AdT = np.ascontiguousarray(A_down64.T).astype(bf)          # [H, R]
    BdT = np.ascontiguousarray(
        (SCALING * np.asarray(B_down, np.float64)).T).astype(bf)  # [R, D]
    WiT_b = np.ascontiguousarray(np.asarray(Wi, np.float32).T).astype(bf)  # [D, H]
    bi32 = np.ascontiguousarray(np.asarray(bi, np.float32))

    in_maps = []
    for c in range(NCORES):
        rows = slice(c * TOK, (c + 1) * TOK)
        in_maps.append(dict(
            xT=np.ascontiguousarray(x2[rows].T).astype(bf),
            base=np.ascontiguousarray(base2[rows]),
            WiT=WiT_b, bi=bi32, AfT=AfT, AdT=AdT, BdT=BdT,
        ))

    nc = _get_nc()
    last_results = run_bass_kernel_spmd(nc, in_maps, core_ids=list(range(NCORES)))
    out = np.concatenate([r["out"] for r in last_results.results], axis=0)
    return np.ascontiguousarray(out.reshape(B, S, D).astype(np.float32))


# revision 23
# speedup vs baseline: 2.0107x; 1.4387x over previous
"""Trainium2 Bass kernel for a BERT-style feed-forward expert with LoRA in/out
adapters and a final no-affine layernorm.

Reference computation (per token t, D=1024, H=4096, R=8):
    lora_up  = (x @ A_up.T) @ B_up.T * s
    hidden   = gelu(x @ Wi.T + bi) + lora_up
    out      = layernorm(base + (hidden @ A_down.T) @ B_down.T * s)

Structural facts exploited:
  * `hidden` is consumed ONLY through the rank-8 down projection, so the
    lora_up contribution collapses algebraically:
        r2 = A_down @ hidden.T
           = A_down @ gelu(z).T + (s * A_down @ B_up @ A_up) @ x.T
    The 8x1024 matrix A_fold = s * A_down @ B_up @ A_up is precomputed on the
    host; B_up never reaches the device.
  * The final output is LN(base + r2.T @ (s*B_down.T)) with the rank-8 term a
    small perturbation of base, so bf16 matmuls are far more than accurate
    enough; the base-add and layernorm run in fp32.

Device dataflow per core (2048 tokens, data-parallel, no collectives):
  * x^T and Wi^T arrive host-pre-transposed so the D contraction sits on SBUF
    partitions; hidden is produced as [H, tok] tiles and consumed immediately
    by the r2 accumulation - never materialized.
  * All rank-8 matmuls (M=8) are column-packed 4-wide into PE column strips
    via tile_position, so 4 of them cost ~1 matmul of time.
  * The PE instruction stream is kept stall-free by emission-time software
    pipelining: each r2 strip group is emitted two H-blocks after the gelus
    it consumes, and stage C (down-proj + base-add + layernorm) of tile t is
    spread one subtile per H-block across tile t+1.
  * rsqrt for layernorm is a Quake-seeded Newton iteration on the DVE -
    keeping Sqrt off the Scalar engine avoids Gelu<->Sqrt ACT-table thrash.
"""

from contextlib import ExitStack

import numpy as np
import ml_dtypes

import concourse.mybir as mybir
import concourse.tile as tile
from concourse import bacc
from concourse.bass_utils import run_bass_kernel_spmd

# Problem shape (hardcoded per contract; kernel.py must be self-contained).
B, S, D, H, R = 4, 4096, 1024, 4096, 8
NCORES = 8
TOK = (B * S) // NCORES      # tokens per core = 2048
TT = 512                     # token tile (matmul moving free dim)
NTT = TOK // TT              # 4 token tiles
P = 128
KO = D // P                  # 8  k-subtiles of the D contraction
HO = H // P                  # 32 hidden chunks
HB = HO // 4                 # 8  blocks of 4 hidden chunks
NSB = TT // P                # 4  token subtiles per tile
SCALING = 16.0 / 8.0         # lora_alpha / lora_r
LN_EPS = 1e-5

BF16 = mybir.dt.bfloat16
F32 = mybir.dt.float32
I32 = mybir.dt.int32

RSQRT_MAGIC = 0x5F3759DF

_NC_CACHE = {}
last_results = None          # test harness reads exec_time_ns from here

# CoreSim does not implement Gelu; sim_check.py overrides this with Tanh to
# validate scheduling/layout. Hardware always runs the real (exact) Gelu.
ACT_FUNC = mybir.ActivationFunctionType.Gelu


def _build_nc():
    nc = bacc.Bacc("TRN2", target_bir_lowering=False, debug=False,
                   num_devices=NCORES)

    xT = nc.dram_tensor("xT", [D, TOK], BF16, kind="ExternalInput").ap()
    base = nc.dram_tensor("base", [TOK, D], F32, kind="ExternalInput").ap()
    WiT = nc.dram_tensor("WiT", [D, H], BF16, kind="ExternalInput").ap()
    bi = nc.dram_tensor("bi", [H], F32, kind="ExternalInput").ap()
    AfT = nc.dram_tensor("AfT", [D, R], BF16, kind="ExternalInput").ap()
    AdT = nc.dram_tensor("AdT", [H, R], BF16, kind="ExternalInput").ap()
    BdT = nc.dram_tensor("BdT", [R, D], BF16, kind="ExternalInput").ap()
    out = nc.dram_tensor("out", [TOK, D], F32, kind="ExternalOutput").ap()

    with tile.TileContext(nc) as tc, ExitStack() as ctx:
        _body(tc, ctx, xT, base, WiT, bi, AfT, AdT, BdT, out)
    nc.compile()
    return nc


def _body(tc, ctx, xT, base, WiT, bi, AfT, AdT, BdT, out):
    nc = tc.nc
    assert nc.vector.BN_STATS_FMAX >= 512

    singles = ctx.enter_context(tc.tile_pool(name="singles", bufs=1))
    gpool = ctx.enter_context(tc.tile_pool(name="gpool", bufs=20))
    rpool = ctx.enter_context(tc.tile_pool(name="rpool", bufs=2))
    bpool = ctx.enter_context(tc.tile_pool(name="bpool", bufs=3))
    ypool = ctx.enter_context(tc.tile_pool(name="ypool", bufs=6))
    opool = ctx.enter_context(tc.tile_pool(name="opool", bufs=3))
    spool = ctx.enter_context(tc.tile_pool(name="spool", bufs=4))
    # PSUM banks: 3 (hidden chunks) + 2 (r2 accumulators) + 2 + 1 (down-proj
    # halves) = 8
    psum_h = ctx.enter_context(tc.tile_pool(name="psum_h", bufs=3, space="PSUM"))
    psum_r2 = ctx.enter_context(tc.tile_pool(name="psum_r2", bufs=2, space="PSUM"))
    psum_d0 = ctx.enter_context(tc.tile_pool(name="psum_d0", bufs=2, space="PSUM"))
    psum_d1 = ctx.enter_context(tc.tile_pool(name="psum_d1", bufs=1, space="PSUM"))

    gelu = ACT_FUNC
    sub = mybir.AluOpType.subtract
    mult = mybir.AluOpType.mult
    add = mybir.AluOpType.add
    shr = mybir.AluOpType.arith_shift_right

    # ---- resident tensors; emission order = DMA priority. The very first
    # compute is main block b0 (ho 0..3): its first weights chunk goes first,
    # then x tile 0 (split across queues), then everything else.
    wiT_sb = singles.tile([P, KO, H], BF16)
    wiT_dram = WiT.rearrange("(ko p) h -> p ko h", p=P)
    xT_sb = singles.tile([P, KO, TOK], BF16)
    xT_dram = xT.rearrange("(ko p) t -> p ko t", p=P)

    nc.sync.dma_start(wiT_sb[:, :, 0:P], wiT_dram[:, :, 0:P])
    nc.sync.dma_start(xT_sb[:, :, 0:TT], xT_dram[:, :, 0:TT])
    nc.sync.dma_start(wiT_sb[:, :, P:512], wiT_dram[:, :, P:512])
    nc.sync.dma_start(wiT_sb[:, :, 512:1024], wiT_dram[:, :, 512:1024])

    bi_sb = singles.tile([P, HO], F32)
    nc.sync.dma_start(bi_sb, bi.rearrange("(ho p) -> p ho", p=P))
    afT_sb = singles.tile([P, KO, R], BF16)
    nc.sync.dma_start(afT_sb, AfT.rearrange("(ko p) r -> p ko r", p=P))
    adT_sb = singles.tile([P, HO, R], BF16)
    nc.sync.dma_start(adT_sb, AdT.rearrange("(ho p) r -> p ho r", p=P))
    bdT_sb = singles.tile([R, D], BF16)
    nc.sync.dma_start(bdT_sb, BdT)

    HC = 512  # H-chunks; chunk hc holds the weights for hidden block hb=hc
    for hc in range(2, H // HC):
        nc.sync.dma_start(wiT_sb[:, :, hc * HC:(hc + 1) * HC],
                          wiT_dram[:, :, hc * HC:(hc + 1) * HC])
        if hc - 1 < NTT:
            nc.sync.dma_start(xT_sb[:, :, (hc - 1) * TT:hc * TT],
                              xT_dram[:, :, (hc - 1) * TT:hc * TT])

    tile_state = {}

    def emit_r2_group(t2, hb2):
        # 4 rank-8 matmuls packed into distinct PE column strips; they run
        # concurrently in the array (separate col groups / XBUSes).
        st = tile_state[t2]
        pr2 = st["pr2"]
        last = hb2 == HB - 1
        for g, ho in st["groups"][hb2]:
            j = ho % 4
            nc.tensor.matmul(pr2[32 * j:32 * j + R, :], adT_sb[:, ho, :], g,
                             start=False, stop=last, tile_position=(0, 32 * j),
                             skip_group_check=True)
        st["groups"][hb2] = None
        if last:
            # combine the 4 column strips -> r2 [8, TT] (bf16 for down-proj),
            # one 128-token slice at a time so the first down-proj matmul can
            # start after ~1/4 of the combine work
            s0 = rpool.tile([R, TT], F32, name="s0")
            r2_sb = rpool.tile([R, TT], BF16, name="r2_sb")
            for sb in range(NSB):
                c = slice(sb * P, (sb + 1) * P)
                nc.vector.tensor_copy(s0[:, c], pr2[0:R, c])
                nc.vector.tensor_add(s0[:, c], s0[:, c], pr2[32:32 + R, c])
                nc.vector.tensor_add(s0[:, c], s0[:, c], pr2[64:64 + R, c])
                nc.vector.tensor_add(r2_sb[:, c], s0[:, c], pr2[96:96 + R, c])
            st["r2"] = r2_sb

    def stage_c_sub(st, sb):
        # down-projection + base add + layernorm stats for one 128-token
        # subtile of token tile st["t"]
        t0 = st["t"] * TT + sb * P
        r2_sb = st["r2"]
        pd0 = psum_d0.tile([P, 512], F32)
        if st["t"] == NTT - 1:
            # the r2 accumulators are drained by now; reusing their slots
            # doubles pd1 buffering exactly where the final tile's stage C
            # would otherwise serialize on it
            pd1 = psum_r2.tile([P, TT], F32, tag="pr2", name="pd1t")[:, 0:512]
        else:
            pd1 = psum_d1.tile([P, 512], F32)
        nc.tensor.matmul(pd0, r2_sb[:, sb * P:(sb + 1) * P],
                         bdT_sb[:, 0:512], start=True, stop=True)
        nc.tensor.matmul(pd1, r2_sb[:, sb * P:(sb + 1) * P],
                         bdT_sb[:, 512:1024], start=True, stop=True)
        bt = bpool.tile([P, D], F32)
        nc.sync.dma_start(bt, base[t0:t0 + P, :])
        y = ypool.tile([P, D], F32)
        nc.vector.tensor_add(y[:, 0:512], bt[:, 0:512], pd0)
        nc.vector.tensor_add(y[:, 512:1024], bt[:, 512:1024], pd1)
        stats = spool.tile([P, 2, nc.vector.BN_STATS_DIM], F32)
        nc.vector.bn_stats(stats[:, 0, :], y[:, 0:512])
        nc.vector.bn_stats(stats[:, 1, :], y[:, 512:1024])
        nc.vector.bn_aggr(st["mvs"][:, sb, :], stats)
        st["ys"].append(y)

    def stage_c_fin(st):
        # rstd = rsqrt(var + eps) for all 4 subtiles at once, DVE-only:
        # Quake-III seed then 2 Newton iterations (error ~4e-6, far below
        # the bf16 matmul error floor).
        mvs = st["mvs"]
        v = spool.tile([P, NSB], F32)
        nc.vector.tensor_scalar_add(v, mvs[:, :, 1], LN_EPS)
        iv = spool.tile([P, NSB], I32)
        nc.vector.tensor_scalar(out=iv, in0=v.bitcast(I32), scalar1=1,
                                scalar2=None, op0=shr)
        nc.vector.tensor_scalar(out=iv, in0=iv, scalar1=-1, scalar2=RSQRT_MAGIC,
                                op0=mult, op1=add)
        r = iv.bitcast(F32)
        tmp = spool.tile([P, NSB], F32)
        for _ in range(2):
            nc.vector.tensor_mul(tmp, v, r)
            nc.vector.tensor_mul(tmp, tmp, r)
            nc.vector.tensor_scalar(out=tmp, in0=tmp, scalar1=-0.5, scalar2=1.5,
                                    op0=mult, op1=add)
            nc.vector.tensor_mul(r, r, tmp)

        nb = None
        if st["t"] == NTT - 1:
            # last tile: normalize on ACT (gelus are done, so no table
            # thrash) as out = y*rstd + (-mean*rstd), freeing the DVE tail
            nb = spool.tile([P, NSB], F32)
            nc.vector.tensor_mul(nb, mvs[:, :, 0], r)
            nc.vector.tensor_scalar_mul(nb, nb, -1.0)
        for sb in range(NSB):
            t0 = st["t"] * TT + sb * P
            o = opool.tile([P, D], F32)
            if nb is not None:
                nc.scalar.activation(o, st["ys"][sb],
                                     mybir.ActivationFunctionType.Identity,
                                     bias=nb[:, sb:sb + 1],
                                     scale=r[:, sb:sb + 1])
            else:
                nc.vector.tensor_scalar(out=o, in0=st["ys"][sb],
                                        scalar1=mvs[:, sb, 0:1],
                                        scalar2=r[:, sb:sb + 1],
                                        op0=sub, op1=mult)
            nc.sync.dma_start(out[t0:t0 + P, :], o)

    def tick(gb):
        # emission-time pipeline: r2 strip groups run 2 blocks behind their
        # gelus; stage C of tile t runs one subtile per block across tile t+1
        g2 = gb - 2
        if g2 >= 0:
            t2, hb2 = divmod(g2, HB)
            if t2 < NTT:
                emit_r2_group(t2, hb2)
        g3 = gb - (HB + 2)
        if g3 >= 0:
            t3, k = divmod(g3, HB)
            if t3 < NTT:
                if k < NSB:
                    stage_c_sub(tile_state[t3], k)
                elif k == NSB:
                    stage_c_fin(tile_state[t3])

    for t in range(NTT):
        tsl = slice(t * TT, (t + 1) * TT)
        # r2[r, tok] accumulates in one PSUM bank split into 4 column strips:
        # the folded lora-up path (8 D-chunks) starts the strips, then the
        # 32 gelu H-chunks accumulate into them.
        st = {"t": t,
              "pr2": psum_r2.tile([P, TT], F32, tag="pr2", name="pr2"),
              "groups": [], "ys": [], "r2": None,
              "mvs": spool.tile([P, NSB, nc.vector.BN_AGGR_DIM], F32,
                                name="mvs")}
        tile_state[t] = st

        for hb in range(HB):
            group = []
            for hj in range(4):
                ho = hb * 4 + hj
                ph = psum_h.tile([P, TT], F32)
                for ko in range(KO):
                    nc.tensor.matmul(ph, wiT_sb[:, ko, ho * P:(ho + 1) * P],
                                     xT_sb[:, ko, tsl],
                                     start=(ko == 0), stop=(ko == KO - 1))
                g = gpool.tile([P, TT], BF16)
                nc.scalar.activation(g, ph, gelu, bias=bi_sb[:, ho:ho + 1],
                                     scale=1.0)
                group.append((g, ho))
            st["groups"].append(group)
            if hb == 0:
                pr2 = st["pr2"]
                for ko in range(KO):
                    j = ko % 4
                    nc.tensor.matmul(pr2[32 * j:32 * j + R, :], afT_sb[:, ko, :],
                                     xT_sb[:, ko, tsl],
                                     start=(ko < 4), stop=False,
                                     tile_position=(0, 32 * j),
                                     skip_group_check=True)
            tick(t * HB + hb)

    for gb in range(NTT * HB, NTT * HB + HB + NSB + 3):
        tick(gb)


def _get_nc():
    if "nc" not in _NC_CACHE:
        _NC_CACHE["nc"] = _build_nc()
    return _NC_CACHE["nc"]


def kernel(x, base_output, Wi, bi, A_up, B_up, A_down, B_down):
    global last_results
    bf = ml_dtypes.bfloat16

    x2 = np.asarray(x, dtype=np.float32).reshape(B * S, D)
    base2 = np.asarray(base_output, dtype=np.float32).reshape(B * S, D)

    A_up64 = np.asarray(A_up, dtype=np.float64)
    B_up64 = np.asarray(B_up, dtype=np.float64)
    A_down64 = np.asarray(A_down, dtype=np.float64)
    A_fold = (SCALING * (A_down64 @ B_up64)) @ A_up64          # [R, D]

    AfT = np.ascontiguousarray(A_fold.T).astype(bf)            # [D, R]
    AdT = np.ascontiguousarray(A_down64.T).astype(bf)          # [H, R]
    BdT = np.ascontiguousarray(
        (SCALING * np.asarray(B_down, np.float64)).T).astype(bf)  # [R, D]
    WiT_b = np.ascontiguousarray(np.asarray(Wi, np.float32).T).astype(bf)  # [D, H]
    bi32 = np.ascontiguousarray(np.asarray(bi, np.float32))

    in_maps = []
    for c in range(NCORES):
        rows = slice(c * TOK, (c + 1) * TOK)
        in_maps.append(dict(
            xT=np.ascontiguousarray(x2[rows].T).astype(bf),
            base=np.ascontiguousarray(base2[rows]),
            WiT=WiT_b, bi=bi32, AfT=AfT, AdT=AdT, BdT=BdT,
        ))

    nc = _get_nc()
    last_results = run_bass_kernel_spmd(nc, in_maps, core_ids=list(range(NCORES)))
    out = np.concatenate([r["out"] for r in last_results.results], axis=0)
    return np.ascontiguousarray(out.reshape(B, S, D).astype(np.float32))


# revision 24
# speedup vs baseline: 2.0264x; 1.0078x over previous
"""Trainium2 Bass kernel for a BERT-style feed-forward expert with LoRA in/out
adapters and a final no-affine layernorm.

Reference computation (per token t, D=1024, H=4096, R=8):
    lora_up  = (x @ A_up.T) @ B_up.T * s
    hidden   = gelu(x @ Wi.T + bi) + lora_up
    out      = layernorm(base + (hidden @ A_down.T) @ B_down.T * s)

Structural facts exploited:
  * `hidden` is consumed ONLY through the rank-8 down projection, so the
    lora_up contribution collapses algebraically:
        r2 = A_down @ hidden.T
           = A_down @ gelu(z).T + (s * A_down @ B_up @ A_up) @ x.T
    The 8x1024 matrix A_fold = s * A_down @ B_up @ A_up is precomputed on the
    host; B_up never reaches the device.
  * The final output is LN(base + r2.T @ (s*B_down.T)) with the rank-8 term a
    small perturbation of base, so bf16 matmuls are far more than accurate
    enough; the base-add and layernorm run in fp32.

Device dataflow per core (2048 tokens, data-parallel, no collectives):
  * x^T and Wi^T arrive host-pre-transposed so the D contraction sits on SBUF
    partitions; hidden is produced as [H, tok] tiles and consumed immediately
    by the r2 accumulation - never materialized.
  * All rank-8 matmuls (M=8) are column-packed 4-wide into PE column strips
    via tile_position, so 4 of them cost ~1 matmul of time.
  * The PE instruction stream is kept stall-free by emission-time software
    pipelining: each r2 strip group is emitted two H-blocks after the gelus
    it consumes, and stage C (down-proj + base-add + layernorm) of tile t is
    spread one subtile per H-block across tile t+1.
  * rsqrt for layernorm is a Quake-seeded Newton iteration on the DVE -
    keeping Sqrt off the Scalar engine avoids Gelu<->Sqrt ACT-table thrash.
"""

from contextlib import ExitStack

import numpy as np
import ml_dtypes

import concourse.mybir as mybir
import concourse.tile as tile
from concourse import bacc
from concourse.bass_utils import run_bass_kernel_spmd

# Problem shape (hardcoded per contract; kernel.py must be self-contained).
B, S, D, H, R = 4, 4096, 1024, 4096, 8
NCORES = 8
TOK = (B * S) // NCORES      # tokens per core = 2048
TT = 512                     # token tile (matmul moving free dim)
NTT = TOK // TT              # 4 token tiles
P = 128
KO = D // P                  # 8  k-subtiles of the D contraction
HO = H // P                  # 32 hidden chunks
HB = HO // 4                 # 8  blocks of 4 hidden chunks
NSB = TT // P                # 4  token subtiles per tile
SCALING = 16.0 / 8.0         # lora_alpha / lora_r
LN_EPS = 1e-5

BF16 = mybir.dt.bfloat16
F32 = mybir.dt.float32
I32 = mybir.dt.int32
F8 = mybir.dt.float8e4
XSC, WSC = 8.0, 32.0          # fp8 pre-scales; 1/(XSC*WSC) folded into gelu

RSQRT_MAGIC = 0x5F3759DF

_NC_CACHE = {}
last_results = None          # test harness reads exec_time_ns from here

# CoreSim does not implement Gelu; sim_check.py overrides this with Tanh to
# validate scheduling/layout. Hardware always runs the real (exact) Gelu.
ACT_FUNC = mybir.ActivationFunctionType.Gelu


def _build_nc():
    nc = bacc.Bacc("TRN2", target_bir_lowering=False, debug=False,
                   num_devices=NCORES)

    xT = nc.dram_tensor("xT", [D, TOK], BF16, kind="ExternalInput").ap()
    base = nc.dram_tensor("base", [TOK, D], F32, kind="ExternalInput").ap()
    WiT = nc.dram_tensor("WiT", [D, H], F8, kind="ExternalInput").ap()
    X8 = nc.dram_tensor("X8", [D, TOK], F8, kind="ExternalInput").ap()
    bi = nc.dram_tensor("bi", [H], F32, kind="ExternalInput").ap()
    AfT = nc.dram_tensor("AfT", [D, R], BF16, kind="ExternalInput").ap()
    AdT = nc.dram_tensor("AdT", [H, R], BF16, kind="ExternalInput").ap()
    BdT = nc.dram_tensor("BdT", [R, D], BF16, kind="ExternalInput").ap()
    out = nc.dram_tensor("out", [TOK, D], F32, kind="ExternalOutput").ap()

    with tile.TileContext(nc) as tc, ExitStack() as ctx:
        _body(tc, ctx, xT, X8, base, WiT, bi, AfT, AdT, BdT, out)
    nc.compile()
    return nc


def _body(tc, ctx, xT, X8, base, WiT, bi, AfT, AdT, BdT, out):
    nc = tc.nc
    assert nc.vector.BN_STATS_FMAX >= 512

    singles = ctx.enter_context(tc.tile_pool(name="singles", bufs=1))
    gpool = ctx.enter_context(tc.tile_pool(name="gpool", bufs=20))
    rpool = ctx.enter_context(tc.tile_pool(name="rpool", bufs=2))
    bpool = ctx.enter_context(tc.tile_pool(name="bpool", bufs=3))
    ypool = ctx.enter_context(tc.tile_pool(name="ypool", bufs=6))
    opool = ctx.enter_context(tc.tile_pool(name="opool", bufs=3))
    spool = ctx.enter_context(tc.tile_pool(name="spool", bufs=4))
    # PSUM banks: 3 (hidden chunks) + 2 (r2 accumulators) + 2 + 1 (down-proj
    # halves) = 8
    psum_h = ctx.enter_context(tc.tile_pool(name="psum_h", bufs=3, space="PSUM"))
    psum_r2 = ctx.enter_context(tc.tile_pool(name="psum_r2", bufs=2, space="PSUM"))
    psum_d0 = ctx.enter_context(tc.tile_pool(name="psum_d0", bufs=2, space="PSUM"))
    psum_d1 = ctx.enter_context(tc.tile_pool(name="psum_d1", bufs=1, space="PSUM"))

    gelu = ACT_FUNC
    sub = mybir.AluOpType.subtract
    mult = mybir.AluOpType.mult
    add = mybir.AluOpType.add
    shr = mybir.AluOpType.arith_shift_right

    # ---- resident tensors; emission order = DMA priority. The very first
    # compute is main block b0 (ho 0..3): its first weights chunk goes first,
    # then x tile 0 (split across queues), then everything else.
    wiT_sb = singles.tile([P, KO, H], F8)
    wiT_dram = WiT.rearrange("(ko p) h -> p ko h", p=P)
    xT_sb = singles.tile([P, KO, TOK], BF16)
    xT_dram = xT.rearrange("(ko p) t -> p ko t", p=P)
    x8_sb = singles.tile([P, KO, TOK], F8)
    x8_dram = X8.rearrange("(ko p) t -> p ko t", p=P)

    nc.sync.dma_start(wiT_sb[:, :, 0:P], wiT_dram[:, :, 0:P])
    nc.sync.dma_start(x8_sb[:, :, 0:TT], x8_dram[:, :, 0:TT])
    nc.sync.dma_start(xT_sb[:, :, 0:TT], xT_dram[:, :, 0:TT])
    nc.sync.dma_start(wiT_sb[:, :, P:512], wiT_dram[:, :, P:512])
    nc.sync.dma_start(wiT_sb[:, :, 512:1024], wiT_dram[:, :, 512:1024])

    bi_sb = singles.tile([P, HO], F32)
    nc.sync.dma_start(bi_sb, bi.rearrange("(ho p) -> p ho", p=P))
    afT_sb = singles.tile([P, KO, R], BF16)
    nc.sync.dma_start(afT_sb, AfT.rearrange("(ko p) r -> p ko r", p=P))
    adT_sb = singles.tile([P, HO, R], BF16)
    nc.sync.dma_start(adT_sb, AdT.rearrange("(ho p) r -> p ho r", p=P))
    bdT_sb = singles.tile([R, D], BF16)
    nc.sync.dma_start(bdT_sb, BdT)

    HC = 512  # H-chunks; chunk hc holds the weights for hidden block hb=hc
    for hc in range(2, H // HC):
        nc.sync.dma_start(wiT_sb[:, :, hc * HC:(hc + 1) * HC],
                          wiT_dram[:, :, hc * HC:(hc + 1) * HC])
        if hc - 1 < NTT:
            nc.sync.dma_start(x8_sb[:, :, (hc - 1) * TT:hc * TT],
                              x8_dram[:, :, (hc - 1) * TT:hc * TT])
            nc.sync.dma_start(xT_sb[:, :, (hc - 1) * TT:hc * TT],
                              xT_dram[:, :, (hc - 1) * TT:hc * TT])

    tile_state = {}

    def emit_r2_group(t2, hb2):
        # 4 rank-8 matmuls packed into distinct PE column strips; they run
        # concurrently in the array (separate col groups / XBUSes).
        st = tile_state[t2]
        pr2 = st["pr2"]
        last = hb2 == HB - 1
        for g, ho in st["groups"][hb2]:
            j = ho % 4
            nc.tensor.matmul(pr2[32 * j:32 * j + R, :], adT_sb[:, ho, :], g,
                             start=False, stop=last, tile_position=(0, 32 * j),
                             skip_group_check=True)
        st["groups"][hb2] = None
        if last:
            # combine the 4 column strips -> r2 [8, TT] (bf16 for down-proj),
            # one 128-token slice at a time so the first down-proj matmul can
            # start after ~1/4 of the combine work
            s0 = rpool.tile([R, TT], F32, name="s0")
            r2_sb = rpool.tile([R, TT], BF16, name="r2_sb")
            for sb in range(NSB):
                c = slice(sb * P, (sb + 1) * P)
                nc.vector.tensor_copy(s0[:, c], pr2[0:R, c])
                nc.vector.tensor_add(s0[:, c], s0[:, c], pr2[32:32 + R, c])
                nc.vector.tensor_add(s0[:, c], s0[:, c], pr2[64:64 + R, c])
                nc.vector.tensor_add(r2_sb[:, c], s0[:, c], pr2[96:96 + R, c])
            st["r2"] = r2_sb

    def stage_c_sub(st, sb):
        # down-projection + base add + layernorm stats for one 128-token
        # subtile of token tile st["t"]
        t0 = st["t"] * TT + sb * P
        r2_sb = st["r2"]
        pd0 = psum_d0.tile([P, 512], F32)
        if st["t"] == NTT - 1:
            # the r2 accumulators are drained by now; reusing their slots
            # doubles pd1 buffering exactly where the final tile's stage C
            # would otherwise serialize on it
            pd1 = psum_r2.tile([P, TT], F32, tag="pr2", name="pd1t")[:, 0:512]
        else:
            pd1 = psum_d1.tile([P, 512], F32)
        nc.tensor.matmul(pd0, r2_sb[:, sb * P:(sb + 1) * P],
                         bdT_sb[:, 0:512], start=True, stop=True)
        nc.tensor.matmul(pd1, r2_sb[:, sb * P:(sb + 1) * P],
                         bdT_sb[:, 512:1024], start=True, stop=True)
        bt = bpool.tile([P, D], F32)
        nc.sync.dma_start(bt, base[t0:t0 + P, :])
        y = ypool.tile([P, D], F32)
        nc.vector.tensor_add(y[:, 0:512], bt[:, 0:512], pd0)
        nc.vector.tensor_add(y[:, 512:1024], bt[:, 512:1024], pd1)
        stats = spool.tile([P, 2, nc.vector.BN_STATS_DIM], F32)
        nc.vector.bn_stats(stats[:, 0, :], y[:, 0:512])
        nc.vector.bn_stats(stats[:, 1, :], y[:, 512:1024])
        nc.vector.bn_aggr(st["mvs"][:, sb, :], stats)
        st["ys"].append(y)

    def stage_c_fin(st):
        # rstd = rsqrt(var + eps) for all 4 subtiles at once, DVE-only:
        # Quake-III seed then 2 Newton iterations (error ~4e-6, far below
        # the bf16 matmul error floor).
        mvs = st["mvs"]
        v = spool.tile([P, NSB], F32)
        nc.vector.tensor_scalar_add(v, mvs[:, :, 1], LN_EPS)
        iv = spool.tile([P, NSB], I32)
        nc.vector.tensor_scalar(out=iv, in0=v.bitcast(I32), scalar1=1,
                                scalar2=None, op0=shr)
        nc.vector.tensor_scalar(out=iv, in0=iv, scalar1=-1, scalar2=RSQRT_MAGIC,
                                op0=mult, op1=add)
        r = iv.bitcast(F32)
        tmp = spool.tile([P, NSB], F32)
        for _ in range(2):
            nc.vector.tensor_mul(tmp, v, r)
            nc.vector.tensor_mul(tmp, tmp, r)
            nc.vector.tensor_scalar(out=tmp, in0=tmp, scalar1=-0.5, scalar2=1.5,
                                    op0=mult, op1=add)
            nc.vector.tensor_mul(r, r, tmp)

        nb = None
        if st["t"] == NTT - 1:
            # last tile: normalize on ACT (gelus are done, so no table
            # thrash) as out = y*rstd + (-mean*rstd), freeing the DVE tail
            nb = spool.tile([P, NSB], F32)
            nc.vector.tensor_mul(nb, mvs[:, :, 0], r)
            nc.vector.tensor_scalar_mul(nb, nb, -1.0)
        for sb in range(NSB):
            t0 = st["t"] * TT + sb * P
            o = opool.tile([P, D], F32)
            if nb is not None:
                nc.scalar.activation(o, st["ys"][sb],
                                     mybir.ActivationFunctionType.Identity,
                                     bias=nb[:, sb:sb + 1],
                                     scale=r[:, sb:sb + 1])
            else:
                nc.vector.tensor_scalar(out=o, in0=st["ys"][sb],
                                        scalar1=mvs[:, sb, 0:1],
                                        scalar2=r[:, sb:sb + 1],
                                        op0=sub, op1=mult)
            nc.sync.dma_start(out[t0:t0 + P, :], o)

    def tick(gb):
        # emission-time pipeline: r2 strip groups run 2 blocks behind their
        # gelus; stage C of tile t runs one subtile per block across tile t+1
        g2 = gb - 2
        if g2 >= 0:
            t2, hb2 = divmod(g2, HB)
            if t2 < NTT:
                emit_r2_group(t2, hb2)
        g3 = gb - (HB + 2)
        if g3 >= 0:
            t3, k = divmod(g3, HB)
            if t3 < NTT:
                if k < NSB:
                    stage_c_sub(tile_state[t3], k)
                elif k == NSB:
                    stage_c_fin(tile_state[t3])

    for t in range(NTT):
        tsl = slice(t * TT, (t + 1) * TT)
        # r2[r, tok] accumulates in one PSUM bank split into 4 column strips:
        # the folded lora-up path (8 D-chunks) starts the strips, then the
        # 32 gelu H-chunks accumulate into them.
        st = {"t": t,
              "pr2": psum_r2.tile([P, TT], F32, tag="pr2", name="pr2"),
              "groups": [], "ys": [], "r2": None,
              "mvs": spool.tile([P, NSB, nc.vector.BN_AGGR_DIM], F32,
                                name="mvs")}
        tile_state[t] = st

        for hb in range(HB):
            group = []
            for hj in range(4):
                ho = hb * 4 + hj
                ph = psum_h.tile([P, TT], F32)
                for k2 in range(KO // 2):
                    nc.tensor.matmul(
                        ph, wiT_sb[:, 2 * k2:2 * k2 + 2, ho * P:(ho + 1) * P],
                        x8_sb[:, 2 * k2:2 * k2 + 2, tsl],
                        start=(k2 == 0), stop=(k2 == KO // 2 - 1),
                        perf_mode=mybir.MatmulPerfMode.DoubleRow)
                g = gpool.tile([P, TT], BF16)
                nc.scalar.activation(g, ph, gelu, bias=bi_sb[:, ho:ho + 1],
                                     scale=1.0 / (XSC * WSC))
                group.append((g, ho))
            st["groups"].append(group)
            if hb == 0:
                pr2 = st["pr2"]
                for ko in range(KO):
                    j = ko % 4
                    nc.tensor.matmul(pr2[32 * j:32 * j + R, :], afT_sb[:, ko, :],
                                     xT_sb[:, ko, tsl],
                                     start=(ko < 4), stop=False,
                                     tile_position=(0, 32 * j),
                                     skip_group_check=True)
            tick(t * HB + hb)

    for gb in range(NTT * HB, NTT * HB + HB + NSB + 3):
        tick(gb)


def _get_nc():
    if "nc" not in _NC_CACHE:
        _NC_CACHE["nc"] = _build_nc()
    return _NC_CACHE["nc"]


def kernel(x, base_output, Wi, bi, A_up, B_up, A_down, B_down):
    global last_results
    bf = ml_dtypes.bfloat16

    x2 = np.asarray(x, dtype=np.float32).reshape(B * S, D)
    base2 = np.asarray(base_output, dtype=np.float32).reshape(B * S, D)

    A_up64 = np.asarray(A_up, dtype=np.float64)
    B_up64 = np.asarray(B_up, dtype=np.float64)
    A_down64 = np.asarray(A_down, dtype=np.float64)
    A_fold = (SCALING * (A_down64 @ B_up64)) @ A_up64          # [R, D]

    AfT = np.ascontiguousarray(A_fold.T).astype(bf)            # [D, R]
    AdT = np.ascontiguousarray(A_down64.T).astype(bf)          # [H, R]
    BdT = np.ascontiguousarray(
        (SCALING * np.asarray(B_down, np.float64)).T).astype(bf)  # [R, D]
    f8 = ml_dtypes.float8_e4m3
    WiT_b = np.ascontiguousarray(WSC * np.asarray(Wi, np.float32).T).astype(f8)
    bi32 = np.ascontiguousarray(np.asarray(bi, np.float32))

    in_maps = []
    for c in range(NCORES):
        rows = slice(c * TOK, (c + 1) * TOK)
        in_maps.append(dict(
            xT=np.ascontiguousarray(x2[rows].T).astype(bf),
            X8=np.ascontiguousarray(XSC * x2[rows].T).astype(f8),
            base=np.ascontiguousarray(base2[rows]),
            WiT=WiT_b, bi=bi32, AfT=AfT, AdT=AdT, BdT=BdT,
        ))

    nc = _get_nc()
    last_results = run_bass_kernel_spmd(nc, in_maps, core_ids=list(range(NCORES)))
    out = np.concatenate([r["out"] for r in last_results.results], axis=0)
    return np.ascontiguousarray(out.reshape(B, S, D).astype(np.float32))
